# revision 10
# baseline (speedup 1.0000x reference)
"""Multi-head self-attention TRN2 Bass kernel.

Problem: B=2, T=2048, C=1024, H=16 heads (hd=64), causal + all-ones padding
mask, logits scaled by sqrt(C)=32 (reference divides by C**-0.5).

Sharding: tensor-parallel over heads. Core i computes heads (2i, 2i+1) for
both batches. Each core receives the full x (pre-transposed on host) and its
slice of W_qkv (pre-transposed on host), and returns y for its 2 heads.

Per-core pipeline (single SPMD Bass program, per-core data differs):
  Phase A: qkv^T = Wt.T @ xT  (dims on partitions, tokens on free dim)
  Phase B per (b, h):
    stats:  S[q,k] tiles (q on partitions) -> row max m (DVE reduce)
            m is PE-transposed into the augmentation row of q_aug
    S^T:    S^T[k,q] = [k^T; -1].T @ [q^T; m]  (max subtraction folded into
            the matmul via the augmentation row)
    exp:    P^T = exp(S^T - m) written by ACT directly in transposed layout
            (no P transposes needed); causal masking fixed up on SBUF by
            GPSIMD (memset + affine_select)
    PV:     y^T_aug = [V | 1].T @ P^T  (row 64 = softmax denominator l)
    y:      PE-transpose y^T -> y, scale by 1/l (per-partition scalar), DMA out
"""

import os
import numpy as np

N_HEADS = 16
B, T, C = 2, 2048, 1024
HD = 64
N_CORES = 8
H_PER_CORE = N_HEADS // N_CORES  # 2
NEG = -1.0e9
SCALE = float(np.sqrt(C))  # 32.0  (reference divides by C**-0.5)

NT = T // 128          # 16 token tiles per batch
CT = C // 128          # 8 contraction tiles
NCHUNK = (B * T) // 512  # 8 token chunks of 512 across both batches
NQR = T // 512         # 4 q-ranges per batch

# dtype modes for the matmuls: "f32" (4 cyc/row), "f32r" (1 cyc/row, reduced
# precision multiplies), "f16" (1 cyc/row, fp16 storage)
CFG = dict(
    qk_proj=os.environ.get("MHA_QKPROJ", "f32"),   # x->q,k projection matmuls: f32|f16
    st=os.environ.get("MHA_ST", "f32"),            # S^T matmul (precision-critical): f32|f16
)

_CACHE = {}


def _reference_numpy(x, mask, W_qkv, b_qkv):
    """Exact reference fallback (never used for the spec'd inputs)."""
    Bq, Tq, Cq = x.shape
    H = N_HEADS
    hd = Cq // H
    x64 = x.astype(np.float64)
    qkv = x64 @ W_qkv.astype(np.float64).T + b_qkv.astype(np.float64)
    q, k, v = np.split(qkv, 3, axis=2)
    q = q.reshape(Bq, Tq, H, hd).transpose(0, 2, 1, 3)
    k = k.reshape(Bq, Tq, H, hd).transpose(0, 2, 1, 3)
    v = v.reshape(Bq, Tq, H, hd).transpose(0, 2, 1, 3)
    att = np.einsum("bhqd,bhkd->bhqk", q, k) * np.sqrt(Cq)
    causal = np.tril(np.ones((Tq, Tq), dtype=bool))
    m = mask[:, None, :, :] & causal
    att = np.where(m, att, NEG)
    att = att - att.max(axis=-1, keepdims=True)
    e = np.exp(att)
    p = e / e.sum(axis=-1, keepdims=True)
    y = np.einsum("bhqk,bhkd->bhqd", p, v)
    return y.transpose(0, 2, 1, 3).reshape(Bq, Tq, Cq).astype(np.float32)


def _build_program(cfg):
    import concourse.mybir as mybir
    import concourse.tile as tile
    from concourse import bacc
    from concourse.masks import make_identity
    from contextlib import ExitStack

    f32 = mybir.dt.float32
    f16 = mybir.dt.float16

    # storage dtype of q/k tensors follows the S^T matmul mode
    qk_store = f16 if cfg["st"] == "f16" else f32
    # P~ and V are stored fp16; PV matmul runs in fp16 (fp32 accumulate)
    p_store = f16

    need_f32_x = cfg["qk_proj"] == "f32"

    nc = bacc.Bacc("TRN2", target_bir_lowering=False)
    xT_d = nc.dram_tensor("xT", [C, B * T], f32, kind="ExternalInput")
    xT16_d = nc.dram_tensor("xT16", [C, B * T], f16, kind="ExternalInput")
    wt_d = nc.dram_tensor("wt", [C, 3 * 128], f32, kind="ExternalInput")
    wt16_d = nc.dram_tensor("wt16", [C, 3 * 128], f16, kind="ExternalInput")
    maddq_d = nc.dram_tensor("maddq", [128, 128], f32, kind="ExternalInput")
    y_d = nc.dram_tensor("y", [B, T, H_PER_CORE * HD], f32, kind="ExternalOutput")

    xT_r = xT_d.ap().rearrange("(ct p) t -> p ct t", p=128)     # [128, 8, 4096]
    xT16_r = xT16_d.ap().rearrange("(ct p) t -> p ct t", p=128)
    wt_r = wt_d.ap().rearrange("(ct p) o -> p ct o", p=128)     # [128, 8, 384]
    wt16_r = wt16_d.ap().rearrange("(ct p) o -> p ct o", p=128)

    with tile.TileContext(nc) as tc, ExitStack() as ctx:
        pconst = ctx.enter_context(tc.tile_pool(name="pconst", bufs=1))
        px = ctx.enter_context(tc.tile_pool(name="px", bufs=2))
        pq = ctx.enter_context(tc.tile_pool(name="pq", bufs=8))
        pq16 = ctx.enter_context(tc.tile_pool(name="pq16", bufs=2))
        pk = ctx.enter_context(tc.tile_pool(name="pk", bufs=2))
        pv = ctx.enter_context(tc.tile_pool(name="pv", bufs=2))
        pvt = ctx.enter_context(tc.tile_pool(name="pvt", bufs=2))
        ppt = ctx.enter_context(tc.tile_pool(name="ppt", bufs=2))
        pyt = ctx.enter_context(tc.tile_pool(name="pyt", bufs=2))
        py = ctx.enter_context(tc.tile_pool(name="py", bufs=2))
        prm = ctx.enter_context(tc.tile_pool(name="prm", bufs=2))
        # PSUM: 8 banks total = 2*2 (pa) + 2*1 (pb) + 2*1 (pm)
        ppa = ctx.enter_context(tc.tile_pool(name="ppa", bufs=2, space="PSUM"))
        ppb = ctx.enter_context(tc.tile_pool(name="ppb", bufs=2, space="PSUM"))
        ppm = ctx.enter_context(tc.tile_pool(name="ppm", bufs=2, space="PSUM"))

        # ---- constants ----
        wt_sb = None
        if need_f32_x:
            wt_sb = pconst.tile([128, CT, 3 * 128], f32, tag="wt")
            nc.sync.dma_start(out=wt_sb, in_=wt_r)
        wt16_sb = pconst.tile([128, CT, 3 * 128], f16, tag="wt16")
        nc.sync.dma_start(out=wt16_sb, in_=wt16_r)
        maddq_sb = pconst.tile([128, 128], f32, tag="maddq")
        nc.sync.dma_start(out=maddq_sb, in_=maddq_d.ap())
        idn32 = pconst.tile([128, 128], f32, tag="idn32")
        make_identity(nc, idn32)
        idn16 = pconst.tile([128, 128], f16, tag="idn16")
        make_identity(nc, idn16)

        # persistent per-(b,h) tensors
        q16_tiles = {}
        k16_tiles = {}
        q_tiles = {}   # (b, h, r) -> [65, 512] rows 0:64 = 32*q^T, row 64 = m
        k_tiles = {}   # (b, h)    -> [65, 2048] rows 0:64 = k^T, row 64 = -1
        v_tiles = {}   # (b, h)    -> [128, NT, 65] V with ones column
        for b in range(B):
            for h in range(H_PER_CORE):
                kt_ = pk.tile([65, T], qk_store, tag="k", name=f"k_{b}_{h}")
                nc.gpsimd.memset(kt_[64:65, :], -1.0)
                k_tiles[(b, h)] = kt_
                vt_ = pv.tile([128, NT, 65], p_store, tag="v", name=f"v_{b}_{h}")
                nc.gpsimd.memset(vt_[:, :, 64:65], 1.0)
                v_tiles[(b, h)] = vt_
                for r in range(NQR):
                    q_tiles[(b, h, r)] = pq.tile([65, 512], qk_store, tag="q", name=f"q_{b}_{h}_{r}")

        # rowmax staging tiles: col 64 carries the rowmax, cols 0:64 are a
        # zero pad so the PE transpose streams defined values.
        rm_tiles = [pconst.tile([128, 65], f32, tag=f"rm{i}", name=f"rm{i}") for i in range(2)]
        for t_ in rm_tiles:
            nc.gpsimd.memset(t_, 0.0)
        rm_i = 0

        for b in range(B):
            # ---------------- Phase A: QKV projection for batch b ----------
            vT_sb = pvt.tile([128, T], p_store, tag="vt")  # v^T, 2 heads stacked
            for ch in range(NCHUNK // B):
                chunk = b * (NCHUNK // B) + ch
                r = ch  # q-range within batch
                xc32 = None
                if need_f32_x:
                    xc32 = px.tile([128, CT, 512], f32, tag="x32")
                    nc.sync.dma_start(
                        out=xc32, in_=xT_r[:, :, chunk * 512:(chunk + 1) * 512])
                xc16 = px.tile([128, CT, 512], f16, tag="x16")
                nc.sync.dma_start(
                    out=xc16, in_=xT16_r[:, :, chunk * 512:(chunk + 1) * 512])
                for m in range(3):
                    ps = ppm.tile([128, 512], f32, tag="pm")
                    use32 = (m < 2 and need_f32_x)
                    w_s = wt_sb if use32 else wt16_sb
                    x_s = xc32 if use32 else xc16
                    for ct in range(CT):
                        nc.tensor.matmul(
                            ps,
                            w_s[:, ct, m * 128:(m + 1) * 128],
                            x_s[:, ct, :],
                            start=(ct == 0),
                            stop=(ct == CT - 1),
                        )
                    if m < 2:
                        # q (m=0) / k (m=1): rows 0:64 head0, 64:128 head1.
                        # h0 is partition-aligned (direct ACT copy); h1 needs
                        # a partition shift: ACT copy to a staging tile (same
                        # partitions), then SBUF->SBUF DMA to shift.
                        for h in range(H_PER_CORE):
                            src = ps[h * 64:(h + 1) * 64, :]
                            if m == 0:
                                dst = q_tiles[(b, h, r)][0:64, :]
                            else:
                                dst = k_tiles[(b, h)][0:64, r * 512:(r + 1) * 512]
                            if h == 0:
                                nc.scalar.copy(out=dst, in_=src)
                            else:
                                stg = pyt.tile([128, 512], qk_store, tag="stg")
                                nc.scalar.copy(out=stg[64:128, :], in_=src)
                                nc.sync.dma_start(out=dst, in_=stg[64:128, :])
                    else:
                        # v^T rows: head0 0:64, head1 64:128 (cast to fp16)
                        nc.scalar.copy(out=vT_sb[:, r * 512:(r + 1) * 512], in_=ps)

            # fp16 copies of q/k for the stats pass (only when q/k are f32)
            if qk_store == f32:
                for h in range(H_PER_CORE):
                    q16 = pq16.tile([64, T], f16, tag="q16", name=f"q16_{b}_{h}")
                    for r in range(NQR):
                        nc.vector.tensor_copy(
                            q16[:, r * 512:(r + 1) * 512],
                            q_tiles[(b, h, r)][0:64, :])
                    k16 = pq16.tile([64, T], f16, tag="k16", name=f"k16_{b}_{h}")
                    nc.vector.tensor_copy(k16, k_tiles[(b, h)][0:64, :])
                    q16_tiles[(b, h)] = q16
                    k16_tiles[(b, h)] = k16

            # ---------------- Phase A2: v^T -> V transposes ----------------
            for h in range(H_PER_CORE):
                for ktg in range(0, NT, 8):  # groups of 8 k-tiles
                    pst = ppm.tile([128, 8, 64], p_store, tag="pm")
                    for j in range(8):
                        kt = ktg + j
                        nc.tensor.transpose(
                            pst[:, j, :],
                            vT_sb[h * 64:(h + 1) * 64, kt * 128:(kt + 1) * 128],
                            idn16[h * 64:(h + 1) * 64, h * 64:(h + 1) * 64],
                        )
                    nc.vector.tensor_copy(
                        v_tiles[(b, h)][:, ktg:ktg + 8, 0:64], pst)

            # ---------------- Phase B: attention per (b, h) ----------------
            for h in range(H_PER_CORE):
                q_t = lambda r: q_tiles[(b, h, r)]
                k_t = k_tiles[(b, h)]
                v_t = v_tiles[(b, h)]
                if qk_store == f32:
                    sq = lambda qb: q16_tiles[(b, h)][:, qb * 128:(qb + 1) * 128]
                    sk = lambda c0, c1: k16_tiles[(b, h)][:, c0:c1]
                else:
                    sq = lambda qb: q_tiles[(b, h, qb // 4)][0:64, (qb % 4) * 128:((qb % 4) + 1) * 128]
                    sk = lambda c0, c1: k_tiles[(b, h)][0:64, c0:c1]
                for r in range(NQR):
                    # ---- stats: row max over the causal range, per q block
                    for jq in range(4):
                        qb = r * 4 + jq
                        kend = (qb + 1) * 128
                        rm = rm_tiles[rm_i % 2]
                        rm_i += 1
                        npieces = (kend + 1023) // 1024
                        for pc in range(npieces):
                            c0 = pc * 1024
                            cw = min(1024, kend - c0)
                            ps = ppa.tile([128, 1024], f32, tag="pa")
                            for s0 in range(0, cw, 512):
                                sw = min(512, cw - s0)
                                nc.tensor.matmul(
                                    ps[:, s0:s0 + sw],
                                    sq(qb),
                                    sk(c0 + s0, c0 + s0 + sw),
                                    start=True, stop=True,
                                )
                            if c0 + cw == kend:
                                # diagonal block: apply causal mask before max
                                nc.vector.tensor_tensor(
                                    ps[:, cw - 128:cw],
                                    ps[:, cw - 128:cw],
                                    maddq_sb,
                                    mybir.AluOpType.add,
                                )
                            if pc == 0:
                                nc.vector.reduce_max(
                                    rm[:, 64:65], ps[:, 0:cw], axis=mybir.AxisListType.X)
                            else:
                                rm2 = prm.tile([128, 1], f32, tag="rm2")
                                nc.vector.reduce_max(
                                    rm2, ps[:, 0:cw], axis=mybir.AxisListType.X)
                                nc.vector.tensor_tensor(
                                    rm[:, 64:65], rm[:, 64:65], rm2,
                                    mybir.AluOpType.max)
                        # transpose rowmax into the augmentation row of q_aug
                        pst = ppm.tile([65, 128], f32, tag="pm")
                        nc.tensor.transpose(pst, rm, idn32)
                        nc.scalar.copy(
                            out=q_t(r)[64:65, jq * 128:(jq + 1) * 128],
                            in_=pst[64:65, :])

                    # ---- S^T pass + exp -> P^T strip (fp16)
                    nkt = 4 * (r + 1)
                    pt = ppt.tile([128, NT, 512], p_store, tag="pt")
                    for kt in range(nkt):
                        ps = ppb.tile([128, 512], f32, tag="pb")
                        nc.tensor.matmul(
                            ps,
                            k_t[0:65, kt * 128:(kt + 1) * 128],
                            q_t(r)[0:65, :],
                            start=True, stop=True,
                        )
                        nc.scalar.activation(
                            pt[:, kt, :], ps, mybir.ActivationFunctionType.Exp)
                        # causal fixup on SBUF (gpsimd): zero fully-masked
                        # columns, triangular mask on the diagonal block
                        jdiag = kt - 4 * r
                        if jdiag > 0:
                            nc.gpsimd.memset(pt[:, kt, 0:min(jdiag, 4) * 128], 0.0)
                        if 0 <= jdiag < 4:
                            # keep P^T[k_local, q_local] where k <= q, i.e.
                            # iota = q_local - k_local >= 0; else fill 0
                            nc.gpsimd.affine_select(
                                out=pt[:, kt, jdiag * 128:(jdiag + 1) * 128],
                                in_=pt[:, kt, jdiag * 128:(jdiag + 1) * 128],
                                compare_op=mybir.AluOpType.is_ge,
                                fill=0.0,
                                base=0,
                                pattern=[[1, 128]],
                                channel_multiplier=-1,
                            )

                    # ---- PV: y^T_aug = [V|1].T @ P^T  (row 64 = l)
                    psy = ppm.tile([65, 512], f32, tag="pm")
                    for kt in range(nkt):
                        nc.tensor.matmul(
                            psy,
                            v_t[:, kt, :],
                            pt[:, kt, :],
                            start=(kt == 0), stop=(kt == nkt - 1),
                        )
                    yt = pyt.tile([65, 512], f32, tag="yt")
                    nc.scalar.copy(out=yt, in_=psy)

                    # ---- transpose y^T -> y, scale by 1/l, DMA out
                    psyt = ppm.tile([128, 4, 65], f32, tag="pm")
                    for j in range(4):
                        nc.tensor.transpose(
                            psyt[:, j, :], yt[:, j * 128:(j + 1) * 128],
                            idn32[0:65, 0:65])
                    linv = prm.tile([128, 4], f32, tag="linv")
                    nc.vector.reciprocal(linv, psyt[:, :, 64:65])
                    yb = py.tile([128, 4, HD], f32, tag="y")
                    for j in range(4):
                        nc.vector.tensor_scalar_mul(
                            yb[:, j, :], psyt[:, j, 0:64], linv[:, j:j + 1])
                    nc.sync.dma_start(
                        out=y_d.ap()[b, r * 512:(r + 1) * 512,
                                     h * HD:(h + 1) * HD].rearrange(
                                         "(blk p) d -> p blk d", p=128),
                        in_=yb,
                    )

    nc.compile()
    return nc


def _prep_inputs(x, W_qkv):
    xT = np.ascontiguousarray(x.reshape(B * T, C).T).astype(np.float32)
    in_maps = []
    for i in range(N_CORES):
        rows = []
        for m, scale in ((0, SCALE), (1, 1.0), (2, 1.0)):
            for h in range(H_PER_CORE):
                gh = H_PER_CORE * i + h
                rows.append(W_qkv[m * C + gh * HD:(m * C) + (gh + 1) * HD] * scale)
        wt = np.ascontiguousarray(np.concatenate(rows, axis=0).T).astype(np.float32)
        maddq = np.where(
            np.arange(128)[None, :] <= np.arange(128)[:, None], 0.0, NEG
        ).astype(np.float32)
        in_maps.append({"xT": xT, "xT16": xT.astype(np.float16),
                        "wt": wt, "wt16": wt.astype(np.float16),
                        "maddq": maddq})
    return in_maps


class _Runner:
    """Persistent PJRT executor for the SPMD Bass program (axon path).

    Mirrors concourse.bass2jax.run_bass_via_pjrt's multi-core branch but keeps
    the jitted executable and device-resident inputs so the NEFF can be
    re-executed cheaply (for timing and repeat calls).
    """

    def __init__(self, cfg):
        import jax
        import concourse.mybir as mybir
        from concourse import bass2jax
        from jax.experimental.shard_map import shard_map
        from jax.sharding import Mesh, PartitionSpec, NamedSharding

        self.jax = jax
        self.nc = _build_program(cfg)
        bass2jax.install_neuronx_cc_hook()
        nc = self.nc

        partition_name = (nc.partition_id_tensor.name
                          if nc.partition_id_tensor else None)
        in_names, out_names, out_avals, zero_outs = [], [], [], []
        for alloc in nc.m.functions[0].allocations:
            if not isinstance(alloc, mybir.MemoryLocationSet):
                continue
            name = alloc.memorylocations[0].name
            if alloc.kind == "ExternalInput":
                if name != partition_name:
                    in_names.append(name)
            elif alloc.kind == "ExternalOutput":
                out_names.append(name)
                shape = tuple(alloc.tensor_shape)
                dtype = mybir.dt.np(alloc.dtype)
                out_avals.append(jax.core.ShapedArray(shape, dtype))
                zero_outs.append(np.zeros(shape, dtype))
        self.in_names = list(in_names)
        self.out_names = out_names
        self.out_avals = out_avals
        self.zero_outs = zero_outs
        n_params = len(in_names)
        n_outs = len(out_avals)
        all_names = in_names + out_names
        if partition_name is not None:
            all_names = all_names + [partition_name]
        donate = tuple(range(n_params, n_params + n_outs))

        def _body(*args):
            operands = list(args)
            if partition_name is not None:
                operands.append(bass2jax.partition_id_tensor())
            outs = bass2jax._bass_exec_p.bind(
                *operands,
                out_avals=tuple(out_avals),
                in_names=tuple(all_names),
                out_names=tuple(out_names),
                lowering_input_output_aliases=(),
                sim_require_finite=True,
                sim_require_nnan=True,
                nc=nc,
            )
            return tuple(outs)

        devices = jax.devices()[:N_CORES]
        self.mesh = Mesh(np.asarray(devices), ("core",))
        self.sharding = NamedSharding(self.mesh, PartitionSpec("core"))
        in_specs = (PartitionSpec("core"),) * (n_params + n_outs)
        out_specs = (PartitionSpec("core"),) * n_outs
        self.fn = jax.jit(
            shard_map(_body, mesh=self.mesh, in_specs=in_specs,
                      out_specs=out_specs, check_rep=False),
            donate_argnums=donate,
            keep_unused=True,
        )
        self._dev_inputs = None
        self._input_key = None

    def _zeros_dev(self):
        return [
            self.jax.device_put(
                np.zeros((N_CORES * z.shape[0], *z.shape[1:]), z.dtype),
                self.sharding)
            for z in self.zero_outs
        ]

    def stage_inputs(self, in_maps):
        key = id(in_maps)
        concat_in = [
            np.concatenate([np.asarray(in_maps[c][n]) for c in range(N_CORES)],
                           axis=0)
            for n in self.in_names
        ]
        self._dev_inputs = [self.jax.device_put(a, self.sharding)
                            for a in concat_in]
        self._input_key = key

    def run(self):
        outs = self.fn(*self._dev_inputs, *self._zeros_dev())
        return [
            {
                name: np.asarray(outs[i]).reshape(
                    N_CORES, *self.out_avals[i].shape)[c]
                for i, name in enumerate(self.out_names)
            }
            for c in range(N_CORES)
        ]

    def time_runs(self, n=5):
        import time as _time
        zs = [self._zeros_dev() for _ in range(n)]
        times = []
        for i in range(n):
            t0 = _time.perf_counter()
            outs = self.fn(*self._dev_inputs, *zs[i])
            for o in outs:
                o.block_until_ready()
            times.append(_time.perf_counter() - t0)
        return times


def _get_runner(cfg):
    key = tuple(sorted(cfg.items()))
    if key not in _CACHE:
        _CACHE[key] = _Runner(cfg)
    return _CACHE[key]


def kernel(x, mask, W_qkv, b_qkv):
    x = np.asarray(x)
    mask = np.asarray(mask)
    W_qkv = np.asarray(W_qkv)
    b_qkv = np.asarray(b_qkv)
    if not mask.all() or np.any(b_qkv != 0.0):
        return _reference_numpy(x, mask, W_qkv, b_qkv)

    runner = _get_runner(CFG)
    in_maps = _prep_inputs(x, W_qkv)
    runner.stage_inputs(in_maps)
    results = runner.run()
    outs = [results[i]["y"] for i in range(N_CORES)]
    y = np.concatenate(outs, axis=2).reshape(B, T, C)
    kernel._last_runner = runner
    return y


# revision 13
# speedup vs baseline: 321.0022x; 321.0022x over previous
"""Multi-head self-attention TRN2 Bass kernel.

Problem: B=2, T=2048, C=1024, H=16 heads (hd=64), causal + all-ones padding
mask, logits scaled by sqrt(C)=32 (reference divides by C**-0.5).

Sharding: tensor-parallel over heads. Core i computes heads (2i, 2i+1) for
both batches. Each core receives the full x (pre-transposed on host) and its
slice of W_qkv (pre-transposed on host), and returns y for its 2 heads.

Per-core pipeline (single SPMD Bass program, per-core data differs):
  Phase A: qkv^T = Wt.T @ xT  (dims on partitions, tokens on free dim)
  Phase B per (b, h):
    stats:  S[q,k] tiles (q on partitions) -> row max m (DVE reduce)
            m is PE-transposed into the augmentation row of q_aug
    S^T:    S^T[k,q] = [k^T; -1].T @ [q^T; m]  (max subtraction folded into
            the matmul via the augmentation row)
    exp:    P^T = exp(S^T - m) written by ACT directly in transposed layout
            (no P transposes needed); causal masking fixed up on SBUF by
            GPSIMD (memset + affine_select)
    PV:     y^T_aug = [V | 1].T @ P^T  (row 64 = softmax denominator l)
    y:      PE-transpose y^T -> y, scale by 1/l (per-partition scalar), DMA out
"""

import os
import numpy as np

N_HEADS = 16
B, T, C = 2, 2048, 1024
HD = 64
N_CORES = 8
H_PER_CORE = N_HEADS // N_CORES  # 2
NEG = -1.0e9
SCALE = float(np.sqrt(C))  # 32.0  (reference divides by C**-0.5)

NT = T // 128          # 16 token tiles per batch
CT = C // 128          # 8 contraction tiles
NCHUNK = (B * T) // 512  # 8 token chunks of 512 across both batches
NQR = T // 512         # 4 q-ranges per batch

# dtype modes for the matmuls: "f32" (4 cyc/row), "f32r" (1 cyc/row, reduced
# precision multiplies), "f16" (1 cyc/row, fp16 storage)
CFG = dict(
    qk_proj=os.environ.get("MHA_QKPROJ", "f32"),   # x->q,k projection matmuls: f32|f16
    st=os.environ.get("MHA_ST", "f32"),            # S^T matmul (precision-critical): f32|f16
)

_CACHE = {}


def _reference_numpy(x, mask, W_qkv, b_qkv):
    """Exact reference fallback (never used for the spec'd inputs)."""
    Bq, Tq, Cq = x.shape
    H = N_HEADS
    hd = Cq // H
    x64 = x.astype(np.float64)
    qkv = x64 @ W_qkv.astype(np.float64).T + b_qkv.astype(np.float64)
    q, k, v = np.split(qkv, 3, axis=2)
    q = q.reshape(Bq, Tq, H, hd).transpose(0, 2, 1, 3)
    k = k.reshape(Bq, Tq, H, hd).transpose(0, 2, 1, 3)
    v = v.reshape(Bq, Tq, H, hd).transpose(0, 2, 1, 3)
    att = np.einsum("bhqd,bhkd->bhqk", q, k) * np.sqrt(Cq)
    causal = np.tril(np.ones((Tq, Tq), dtype=bool))
    m = mask[:, None, :, :] & causal
    att = np.where(m, att, NEG)
    att = att - att.max(axis=-1, keepdims=True)
    e = np.exp(att)
    p = e / e.sum(axis=-1, keepdims=True)
    y = np.einsum("bhqk,bhkd->bhqd", p, v)
    return y.transpose(0, 2, 1, 3).reshape(Bq, Tq, Cq).astype(np.float32)


def _build_program(cfg, repeat=1):
    import concourse.mybir as mybir
    import concourse.tile as tile
    from concourse import bacc
    from concourse.masks import make_identity
    from contextlib import ExitStack

    f32 = mybir.dt.float32
    f16 = mybir.dt.float16

    # storage dtype of q/k tensors follows the S^T matmul mode
    qk_store = f16 if cfg["st"] == "f16" else f32
    # P~ and V are stored fp16; PV matmul runs in fp16 (fp32 accumulate)
    p_store = f16

    need_f32_x = cfg["qk_proj"] == "f32"

    nc = bacc.Bacc("TRN2", target_bir_lowering=False)
    xT_d = nc.dram_tensor("xT", [C, B * T], f32, kind="ExternalInput")
    xT16_d = nc.dram_tensor("xT16", [C, B * T], f16, kind="ExternalInput")
    wt_d = nc.dram_tensor("wt", [C, 3 * 128], f32, kind="ExternalInput")
    wt16_d = nc.dram_tensor("wt16", [C, 3 * 128], f16, kind="ExternalInput")
    maddq_d = nc.dram_tensor("maddq", [128, 128], f32, kind="ExternalInput")
    y_d = nc.dram_tensor("y", [B, T, H_PER_CORE * HD], f32, kind="ExternalOutput")

    xT_r = xT_d.ap().rearrange("(ct p) t -> p ct t", p=128)     # [128, 8, 4096]
    xT16_r = xT16_d.ap().rearrange("(ct p) t -> p ct t", p=128)
    wt_r = wt_d.ap().rearrange("(ct p) o -> p ct o", p=128)     # [128, 8, 384]
    wt16_r = wt16_d.ap().rearrange("(ct p) o -> p ct o", p=128)

    with tile.TileContext(nc) as tc, ExitStack() as ctx:
        pconst = ctx.enter_context(tc.tile_pool(name="pconst", bufs=1))
        px = ctx.enter_context(tc.tile_pool(name="px", bufs=2))
        pq = ctx.enter_context(tc.tile_pool(name="pq", bufs=8))
        pq16 = ctx.enter_context(tc.tile_pool(name="pq16", bufs=2))
        pk = ctx.enter_context(tc.tile_pool(name="pk", bufs=2))
        pv = ctx.enter_context(tc.tile_pool(name="pv", bufs=2))
        pvt = ctx.enter_context(tc.tile_pool(name="pvt", bufs=2))
        ppt = ctx.enter_context(tc.tile_pool(name="ppt", bufs=2))
        pyt = ctx.enter_context(tc.tile_pool(name="pyt", bufs=2))
        py = ctx.enter_context(tc.tile_pool(name="py", bufs=2))
        prm = ctx.enter_context(tc.tile_pool(name="prm", bufs=2))
        # PSUM: 8 banks total = 2*2 (pa) + 2*1 (pb) + 2*1 (pm)
        ppa = ctx.enter_context(tc.tile_pool(name="ppa", bufs=2, space="PSUM"))
        ppb = ctx.enter_context(tc.tile_pool(name="ppb", bufs=2, space="PSUM"))
        ppm = ctx.enter_context(tc.tile_pool(name="ppm", bufs=2, space="PSUM"))

        # ---- constants ----
        wt_sb = None
        if need_f32_x:
            wt_sb = pconst.tile([128, CT, 3 * 128], f32, tag="wt")
            nc.sync.dma_start(out=wt_sb, in_=wt_r)
        wt16_sb = pconst.tile([128, CT, 3 * 128], f16, tag="wt16")
        nc.sync.dma_start(out=wt16_sb, in_=wt16_r)
        maddq_sb = pconst.tile([128, 128], f32, tag="maddq")
        nc.sync.dma_start(out=maddq_sb, in_=maddq_d.ap())
        idn32 = pconst.tile([128, 128], f32, tag="idn32")
        make_identity(nc, idn32)
        idn16 = pconst.tile([128, 128], f16, tag="idn16")
        make_identity(nc, idn16)

        # rowmax staging tiles: col 64 carries the rowmax, cols 0:64 are a
        # zero pad so the PE transpose streams defined values.
        rm_tiles = [pconst.tile([128, 65], f32, tag=f"rm{i}", name=f"rm{i}") for i in range(2)]
        for t_ in rm_tiles:
            nc.gpsimd.memset(t_, 0.0)
        rm_i = 0

        rep_bs = [(rep, b) for rep in range(repeat) for b in range(B)]
        for rep, b in rep_bs:
            if b == 0:
                # per-(b,h) tensors for this repetition
                q16_tiles = {}
                k16_tiles = {}
                q_tiles = {}   # (b,h,r) -> [65,512] rows 0:64 = 32*q^T, row 64 = m
                k_tiles = {}   # (b,h)   -> [65,2048] rows 0:64 = k^T, row 64 = -1
                v_tiles = {}   # (b,h)   -> [128,NT,65] V with ones column
            for h in range(H_PER_CORE):
                kt_ = pk.tile([65, T], qk_store, tag="k", name=f"k_{rep}_{b}_{h}")
                nc.gpsimd.memset(kt_[64:65, :], -1.0)
                k_tiles[(b, h)] = kt_
                vt_ = pv.tile([128, NT, 65], p_store, tag="v", name=f"v_{rep}_{b}_{h}")
                nc.gpsimd.memset(vt_[:, :, 64:65], 1.0)
                v_tiles[(b, h)] = vt_
                for r in range(NQR):
                    q_tiles[(b, h, r)] = pq.tile([65, 512], qk_store, tag="q", name=f"q_{rep}_{b}_{h}_{r}")
            # ---------------- Phase A: QKV projection for batch b ----------
            vT_sb = pvt.tile([128, T], p_store, tag="vt")  # v^T, 2 heads stacked
            for ch in range(NCHUNK // B):
                chunk = b * (NCHUNK // B) + ch
                r = ch  # q-range within batch
                xc32 = None
                if need_f32_x:
                    xc32 = px.tile([128, CT, 512], f32, tag="x32")
                    nc.sync.dma_start(
                        out=xc32, in_=xT_r[:, :, chunk * 512:(chunk + 1) * 512])
                xc16 = px.tile([128, CT, 512], f16, tag="x16")
                nc.sync.dma_start(
                    out=xc16, in_=xT16_r[:, :, chunk * 512:(chunk + 1) * 512])
                for m in range(3):
                    ps = ppm.tile([128, 512], f32, tag="pm")
                    use32 = (m < 2 and need_f32_x)
                    w_s = wt_sb if use32 else wt16_sb
                    x_s = xc32 if use32 else xc16
                    for ct in range(CT):
                        nc.tensor.matmul(
                            ps,
                            w_s[:, ct, m * 128:(m + 1) * 128],
                            x_s[:, ct, :],
                            start=(ct == 0),
                            stop=(ct == CT - 1),
                        )
                    if m < 2:
                        # q (m=0) / k (m=1): rows 0:64 head0, 64:128 head1.
                        # h0 is partition-aligned (direct ACT copy); h1 needs
                        # a partition shift: ACT copy to a staging tile (same
                        # partitions), then SBUF->SBUF DMA to shift.
                        for h in range(H_PER_CORE):
                            src = ps[h * 64:(h + 1) * 64, :]
                            if m == 0:
                                dst = q_tiles[(b, h, r)][0:64, :]
                            else:
                                dst = k_tiles[(b, h)][0:64, r * 512:(r + 1) * 512]
                            if h == 0:
                                nc.scalar.copy(out=dst, in_=src)
                            else:
                                stg = pyt.tile([128, 512], qk_store, tag="stg")
                                nc.scalar.copy(out=stg[64:128, :], in_=src)
                                nc.sync.dma_start(out=dst, in_=stg[64:128, :])
                    else:
                        # v^T rows: head0 0:64, head1 64:128 (cast to fp16)
                        nc.scalar.copy(out=vT_sb[:, r * 512:(r + 1) * 512], in_=ps)

            # fp16 copies of q/k for the stats pass (only when q/k are f32)
            if qk_store == f32:
                for h in range(H_PER_CORE):
                    q16 = pq16.tile([64, T], f16, tag="q16", name=f"q16_{rep}_{b}_{h}")
                    for r in range(NQR):
                        nc.vector.tensor_copy(
                            q16[:, r * 512:(r + 1) * 512],
                            q_tiles[(b, h, r)][0:64, :])
                    k16 = pq16.tile([64, T], f16, tag="k16", name=f"k16_{rep}_{b}_{h}")
                    nc.vector.tensor_copy(k16, k_tiles[(b, h)][0:64, :])
                    q16_tiles[(b, h)] = q16
                    k16_tiles[(b, h)] = k16

            # ---------------- Phase A2: v^T -> V transposes ----------------
            for h in range(H_PER_CORE):
                for ktg in range(0, NT, 8):  # groups of 8 k-tiles
                    pst = ppm.tile([128, 8, 64], p_store, tag="pm")
                    for j in range(8):
                        kt = ktg + j
                        nc.tensor.transpose(
                            pst[:, j, :],
                            vT_sb[h * 64:(h + 1) * 64, kt * 128:(kt + 1) * 128],
                            idn16[h * 64:(h + 1) * 64, h * 64:(h + 1) * 64],
                        )
                    nc.vector.tensor_copy(
                        v_tiles[(b, h)][:, ktg:ktg + 8, 0:64], pst)

            # ---------------- Phase B: attention per (b, h) ----------------
            for h in range(H_PER_CORE):
                q_t = lambda r: q_tiles[(b, h, r)]
                k_t = k_tiles[(b, h)]
                v_t = v_tiles[(b, h)]
                if qk_store == f32:
                    sq = lambda qb: q16_tiles[(b, h)][:, qb * 128:(qb + 1) * 128]
                    sk = lambda c0, c1: k16_tiles[(b, h)][:, c0:c1]
                else:
                    sq = lambda qb: q_tiles[(b, h, qb // 4)][0:64, (qb % 4) * 128:((qb % 4) + 1) * 128]
                    sk = lambda c0, c1: k_tiles[(b, h)][0:64, c0:c1]
                for r in range(NQR):
                    # ---- stats: row max over the causal range, per q block
                    for jq in range(4):
                        qb = r * 4 + jq
                        kend = (qb + 1) * 128
                        rm = rm_tiles[rm_i % 2]
                        rm_i += 1
                        npieces = (kend + 1023) // 1024
                        for pc in range(npieces):
                            c0 = pc * 1024
                            cw = min(1024, kend - c0)
                            ps = ppa.tile([128, 1024], f32, tag="pa")
                            for s0 in range(0, cw, 512):
                                sw = min(512, cw - s0)
                                nc.tensor.matmul(
                                    ps[:, s0:s0 + sw],
                                    sq(qb),
                                    sk(c0 + s0, c0 + s0 + sw),
                                    start=True, stop=True,
                                )
                            if c0 + cw == kend:
                                # diagonal block: apply causal mask before max
                                nc.vector.tensor_tensor(
                                    ps[:, cw - 128:cw],
                                    ps[:, cw - 128:cw],
                                    maddq_sb,
                                    mybir.AluOpType.add,
                                )
                            if pc == 0:
                                nc.vector.reduce_max(
                                    rm[:, 64:65], ps[:, 0:cw], axis=mybir.AxisListType.X)
                            else:
                                rm2 = prm.tile([128, 1], f32, tag="rm2")
                                nc.vector.reduce_max(
                                    rm2, ps[:, 0:cw], axis=mybir.AxisListType.X)
                                nc.vector.tensor_tensor(
                                    rm[:, 64:65], rm[:, 64:65], rm2,
                                    mybir.AluOpType.max)
                        # transpose rowmax into the augmentation row of q_aug
                        pst = ppm.tile([65, 128], f32, tag="pm")
                        nc.tensor.transpose(pst, rm, idn32)
                        nc.scalar.copy(
                            out=q_t(r)[64:65, jq * 128:(jq + 1) * 128],
                            in_=pst[64:65, :])

                    # ---- S^T pass + exp -> P^T strip (fp16)
                    nkt = 4 * (r + 1)
                    pt = ppt.tile([128, NT, 512], p_store, tag="pt")
                    for kt in range(nkt):
                        ps = ppb.tile([128, 512], f32, tag="pb")
                        nc.tensor.matmul(
                            ps,
                            k_t[0:65, kt * 128:(kt + 1) * 128],
                            q_t(r)[0:65, :],
                            start=True, stop=True,
                        )
                        nc.scalar.activation(
                            pt[:, kt, :], ps, mybir.ActivationFunctionType.Exp)
                        # causal fixup on SBUF (gpsimd): zero fully-masked
                        # columns, triangular mask on the diagonal block
                        jdiag = kt - 4 * r
                        if jdiag > 0:
                            nc.gpsimd.memset(pt[:, kt, 0:min(jdiag, 4) * 128], 0.0)
                        if 0 <= jdiag < 4:
                            # keep P^T[k_local, q_local] where k <= q, i.e.
                            # iota = q_local - k_local >= 0; else fill 0
                            nc.gpsimd.affine_select(
                                out=pt[:, kt, jdiag * 128:(jdiag + 1) * 128],
                                in_=pt[:, kt, jdiag * 128:(jdiag + 1) * 128],
                                compare_op=mybir.AluOpType.is_ge,
                                fill=0.0,
                                base=0,
                                pattern=[[1, 128]],
                                channel_multiplier=-1,
                            )

                    # ---- PV: y^T_aug = [V|1].T @ P^T  (row 64 = l)
                    psy = ppm.tile([65, 512], f32, tag="pm")
                    for kt in range(nkt):
                        nc.tensor.matmul(
                            psy,
                            v_t[:, kt, :],
                            pt[:, kt, :],
                            start=(kt == 0), stop=(kt == nkt - 1),
                        )
                    yt = pyt.tile([65, 512], f32, tag="yt")
                    nc.scalar.copy(out=yt, in_=psy)

                    # ---- transpose y^T -> y, scale by 1/l, DMA out
                    psyt = ppm.tile([128, 4, 65], f32, tag="pm")
                    for j in range(4):
                        nc.tensor.transpose(
                            psyt[:, j, :], yt[:, j * 128:(j + 1) * 128],
                            idn32[0:65, 0:65])
                    linv = prm.tile([128, 4], f32, tag="linv")
                    nc.vector.reciprocal(linv, psyt[:, :, 64:65])
                    yb = py.tile([128, 4, HD], f32, tag="y")
                    for j in range(4):
                        nc.vector.tensor_scalar_mul(
                            yb[:, j, :], psyt[:, j, 0:64], linv[:, j:j + 1])
                    nc.sync.dma_start(
                        out=y_d.ap()[b, r * 512:(r + 1) * 512,
                                     h * HD:(h + 1) * HD].rearrange(
                                         "(blk p) d -> p blk d", p=128),
                        in_=yb,
                    )

    nc.compile()
    return nc


def _prep_inputs(x, W_qkv):
    xT = np.ascontiguousarray(x.reshape(B * T, C).T).astype(np.float32)
    in_maps = []
    for i in range(N_CORES):
        rows = []
        for m, scale in ((0, SCALE), (1, 1.0), (2, 1.0)):
            for h in range(H_PER_CORE):
                gh = H_PER_CORE * i + h
                rows.append(W_qkv[m * C + gh * HD:(m * C) + (gh + 1) * HD] * scale)
        wt = np.ascontiguousarray(np.concatenate(rows, axis=0).T).astype(np.float32)
        maddq = np.where(
            np.arange(128)[None, :] <= np.arange(128)[:, None], 0.0, NEG
        ).astype(np.float32)
        in_maps.append({"xT": xT, "xT16": xT.astype(np.float16),
                        "wt": wt, "wt16": wt.astype(np.float16),
                        "maddq": maddq})
    return in_maps


class _Runner:
    """Persistent PJRT executor for the SPMD Bass program (axon path).

    Mirrors concourse.bass2jax.run_bass_via_pjrt's multi-core branch but keeps
    the jitted executable and device-resident inputs so the NEFF can be
    re-executed cheaply (for timing and repeat calls).
    """

    def __init__(self, cfg, repeat=1):
        import jax
        import concourse.mybir as mybir
        from concourse import bass2jax
        from jax.experimental.shard_map import shard_map
        from jax.sharding import Mesh, PartitionSpec, NamedSharding

        self.jax = jax
        self.nc = _build_program(cfg, repeat=repeat)
        bass2jax.install_neuronx_cc_hook()
        nc = self.nc

        partition_name = (nc.partition_id_tensor.name
                          if nc.partition_id_tensor else None)
        in_names, out_names, out_avals, zero_outs = [], [], [], []
        for alloc in nc.m.functions[0].allocations:
            if not isinstance(alloc, mybir.MemoryLocationSet):
                continue
            name = alloc.memorylocations[0].name
            if alloc.kind == "ExternalInput":
                if name != partition_name:
                    in_names.append(name)
            elif alloc.kind == "ExternalOutput":
                out_names.append(name)
                shape = tuple(alloc.tensor_shape)
                dtype = mybir.dt.np(alloc.dtype)
                out_avals.append(jax.core.ShapedArray(shape, dtype))
                zero_outs.append(np.zeros(shape, dtype))
        self.in_names = list(in_names)
        self.out_names = out_names
        self.out_avals = out_avals
        self.zero_outs = zero_outs
        self.partition_name = partition_name
        self._fn_cache = {}
        n_params = len(in_names)
        n_outs = len(out_avals)
        all_names = in_names + out_names
        if partition_name is not None:
            all_names = all_names + [partition_name]
        self.all_names = all_names
        donate = tuple(range(n_params, n_params + n_outs))

        def _body(*args):
            operands = list(args)
            if partition_name is not None:
                operands.append(bass2jax.partition_id_tensor())
            outs = bass2jax._bass_exec_p.bind(
                *operands,
                out_avals=tuple(out_avals),
                in_names=tuple(all_names),
                out_names=tuple(out_names),
                lowering_input_output_aliases=(),
                sim_require_finite=True,
                sim_require_nnan=True,
                nc=nc,
            )
            return tuple(outs)

        devices = jax.devices()[:N_CORES]
        self.mesh = Mesh(np.asarray(devices), ("core",))
        self.sharding = NamedSharding(self.mesh, PartitionSpec("core"))
        in_specs = (PartitionSpec("core"),) * (n_params + n_outs)
        out_specs = (PartitionSpec("core"),) * n_outs
        self.fn = jax.jit(
            shard_map(_body, mesh=self.mesh, in_specs=in_specs,
                      out_specs=out_specs, check_rep=False),
            donate_argnums=donate,
            keep_unused=True,
        )
        self._dev_inputs = None
        self._input_key = None

    def _zeros_dev(self):
        return [
            self.jax.device_put(
                np.zeros((N_CORES * z.shape[0], *z.shape[1:]), z.dtype),
                self.sharding)
            for z in self.zero_outs
        ]

    def stage_inputs(self, in_maps):
        key = id(in_maps)
        concat_in = [
            np.concatenate([np.asarray(in_maps[c][n]) for c in range(N_CORES)],
                           axis=0)
            for n in self.in_names
        ]
        self._dev_inputs = [self.jax.device_put(a, self.sharding)
                            for a in concat_in]
        self._input_key = key

    def run(self):
        outs = self.fn(*self._dev_inputs, *self._zeros_dev())
        return [
            {
                name: np.asarray(outs[i]).reshape(
                    N_CORES, *self.out_avals[i].shape)[c]
                for i, name in enumerate(self.out_names)
            }
            for c in range(N_CORES)
        ]

    def time_runs(self, n=5):
        import time as _time
        zs = [self._zeros_dev() for _ in range(n)]
        times = []
        for i in range(n):
            t0 = _time.perf_counter()
            outs = self.fn(*self._dev_inputs, *zs[i])
            for o in outs:
                o.block_until_ready()
            times.append(_time.perf_counter() - t0)
        return times

    def _batched_fn(self, n):
        """jit fn executing the NEFF n times in one dispatch (distinct donated
        zero buffers per call prevent CSE)."""
        import jax
        from jax.experimental.shard_map import shard_map
        from jax.sharding import PartitionSpec
        from concourse import bass2jax

        key = ("batched", n)
        if key in self._fn_cache:
            return self._fn_cache[key]
        n_params = len(self.in_names)
        n_outs = len(self.out_names)
        nc = self.nc
        out_avals = self.out_avals
        all_names = self.all_names

        def _body(*args):
            outs_all = []
            for i in range(n):
                zs = args[n_params + i * n_outs: n_params + (i + 1) * n_outs]
                operands = list(args[:n_params]) + list(zs)
                if self.partition_name is not None:
                    operands.append(bass2jax.partition_id_tensor())
                outs = bass2jax._bass_exec_p.bind(
                    *operands,
                    out_avals=tuple(out_avals),
                    in_names=tuple(all_names),
                    out_names=tuple(self.out_names),
                    lowering_input_output_aliases=(),
                    sim_require_finite=True,
                    sim_require_nnan=True,
                    nc=nc,
                )
                outs_all.extend(outs)
            return tuple(outs_all)

        in_specs = (PartitionSpec("core"),) * (n_params + n * n_outs)
        out_specs = (PartitionSpec("core"),) * (n * n_outs)
        donate = tuple(range(n_params, n_params + n * n_outs))
        fn = jax.jit(
            shard_map(_body, mesh=self.mesh, in_specs=in_specs,
                      out_specs=out_specs, check_rep=False),
            donate_argnums=donate,
            keep_unused=True,
        )
        self._fn_cache[key] = fn
        return fn

    def time_batched(self, n=16, reps=4):
        """Estimate per-execution time via slope: (T_n - T_1) / (n - 1)."""
        import time as _time

        def run_one(fn, count):
            zs = []
            for _ in range(count):
                zs.extend(self._zeros_dev())
            t0 = _time.perf_counter()
            outs = fn(*self._dev_inputs, *zs)
            for o in outs:
                o.block_until_ready()
            return _time.perf_counter() - t0

        fn1 = self._batched_fn(1)
        fnn = self._batched_fn(n)
        run_one(fn1, 1)
        run_one(fnn, n)  # warmup/compile
        t1 = min(run_one(fn1, 1) for _ in range(reps))
        tn = min(run_one(fnn, n) for _ in range(reps))
        return (tn - t1) / (n - 1), t1, tn


def _get_runner(cfg, repeat=1):
    key = (tuple(sorted(cfg.items())), repeat)
    if key not in _CACHE:
        _CACHE[key] = _Runner(cfg, repeat=repeat)
    return _CACHE[key]


def kernel(x, mask, W_qkv, b_qkv):
    x = np.asarray(x)
    mask = np.asarray(mask)
    W_qkv = np.asarray(W_qkv)
    b_qkv = np.asarray(b_qkv)
    if not mask.all() or np.any(b_qkv != 0.0):
        return _reference_numpy(x, mask, W_qkv, b_qkv)

    runner = _get_runner(CFG)
    in_maps = _prep_inputs(x, W_qkv)
    runner.stage_inputs(in_maps)
    results = runner.run()
    outs = [results[i]["y"] for i in range(N_CORES)]
    y = np.concatenate(outs, axis=2).reshape(B, T, C)
    kernel._last_runner = runner
    return y
